# revision 6
# baseline (speedup 1.0000x reference)
"""Bit2Num dequantization kernel for Trainium2 (8 NeuronCores, SPMD).

Reference op: x [1024, 65536] of {0.0, 1.0} f32, B=4.
  bits = x.reshape(1024, 16384, 4)
  out[b, n] = (8*bits[b,n,0] + 4*bits[b,n,1] + 2*bits[b,n,2] + bits[b,n,3] + 0.5) / 16

Sharding: pure data-parallel over batch — 128 rows per core (= 128 SBUF
partitions). Per core: 32 MB f32 in, 16K outputs/row.

The stream is SDMA-engine-aggregate bound (~435-460 GB/s/core, counting
the larger side of each transfer), so total engine-side bytes are what
matter:
  in : 32 MB f32 HBM read (irreducible — the engine cost is the f32
       side whether or not the SBUF side is cast down)
  out: 4 MB bf16 (output staged as bf16 in DRAM — every output value is
       (2k+1)/32, k<16, needing <=5 significand bits, exact in bf16;
       the host widens back to f32 losslessly with astype)
=> ~37.8 MB => ~85-90 us stream window.

Loads ride the sync HWDGE ring (qSPDynamicHW) as plain f32: SWDGE
(gpsimd) loads were measured to strand a single-engine straggler —
SDMA engine 15 runs ~18% slow under SWDGE (descriptor-ring AXI port
contention, a documented TRN2 erratum), and its accumulated backlog
serialized the last ~17 us of the stream at ~11 GB/s. HWDGE has no
SBUF descriptor ring, so all 16 engines drain evenly. Stores ride the
separate scalar HWDGE ring (qActDynamicHW).

Per-core kernel: 18 column segments (14 x 4096 cols + 4 x 2048). Each
segment is one compute chain: 2 fused scalar_tensor_tensor ops on DVE
(pairwise fold: p = 2*x_even + x_odd over pairs, then w = 4*p_even +
p_odd = 8a+4b+2c+d, bf16 out — exact), then the affine (w/16 + 1/32)
on ACT writing the bf16 output tile.
"""

import numpy as np

import concourse.bacc as bacc
import concourse.bass as bass
import concourse.mybir as mybir
from concourse.bass_utils import run_bass_kernel_spmd
from concourse.tile import TileContext

N_CORES = 8
BATCH = 1024
COLS = 65536
B_BITS = 4
ROWS = BATCH // N_CORES          # 128 rows per core == SBUF partition count
OUT_COLS = COLS // B_BITS        # 16384

F32 = mybir.dt.float32
BF16 = mybir.dt.bfloat16
MULT = mybir.AluOpType.mult
ADD = mybir.AluOpType.add

# Input-column segment sizes. 4096-col segments (16 KB/partition f32)
# keep the load pipeline fine-grained. The tail segments are 2048 cols:
# DVE has a ~0.4 us fixed cost per op, so segments below ~2000 cols
# take longer to compute than to arrive and pile up backlog at the
# stream end — the taper must not go finer than this.
SEGMENTS = [4096] * 14 + [2048] * 4
assert sum(SEGMENTS) == COLS


def _build_nc() -> bass.Bass:
    # Bacc (not plain Bass): its compile() pipeline runs
    # generate_event_semaphores, which splits multi-wait sync conditions —
    # TRN2 DMA instructions accept at most one wait.
    nc = bacc.Bacc(None, target_bir_lowering=False)
    x = nc.dram_tensor("x", [ROWS, COLS], F32, kind="ExternalInput")
    out = nc.dram_tensor("out", [ROWS, OUT_COLS], BF16, kind="ExternalOutput")

    with TileContext(nc) as tc:
        with (
            tc.tile_pool(name="xin", bufs=6) as xpool,
            tc.tile_pool(name="work", bufs=3) as wpool,
            tc.tile_pool(name="oout", bufs=4) as opool,
        ):
            col = 0
            for seg_c in SEGMENTS:
                g = seg_c // B_BITS
                g0 = col // B_BITS
                xt = xpool.tile([ROWS, seg_c], F32, tag="xt")
                # HWDGE in-DMA (sync ring, qSPDynamicHW), plain f32.
                nc.sync.dma_start(out=xt[:, :], in_=x[:, col:col + seg_c])
                col += seg_c

                # Pairwise fold: view the segment as pairs; one stt makes
                # p = [u0, v0, u1, v1, ...] (p[2m] = 2a+b, p[2m+1] = 2c+d),
                # a second folds pairs of p into w = 4u+v = 8a+4b+2c+d.
                # Two DVE ops per segment instead of three — same element
                # count, one less ~0.4 us fixed cost, chain depth 2. DVE
                # runs these in 1x mode (strided reads) at the same rate
                # for f32 and bf16; intermediates are bf16 (<=15, exact).
                x2 = xt[:, :].rearrange("p (h k) -> p h k", k=2)
                p = wpool.tile([ROWS, 2 * g], BF16, tag="p")
                w = wpool.tile([ROWS, g], BF16, tag="w")
                ot = opool.tile([ROWS, g], BF16, tag="ot")

                nc.vector.scalar_tensor_tensor(
                    out=p[:, :], in0=x2[:, :, 0], scalar=2.0,
                    in1=x2[:, :, 1], op0=MULT, op1=ADD,
                )
                p2 = p[:, :].rearrange("p (h k) -> p h k", k=2)
                nc.vector.scalar_tensor_tensor(
                    out=w[:, :], in0=p2[:, :, 0], scalar=4.0,
                    in1=p2[:, :, 1], op0=MULT, op1=ADD,
                )
                # ot = (w + 0.5) / 16 = w/16 + 1/32, computed fp32-internal
                # on ACT, written bf16 (exact).
                nc.scalar.activation(
                    out=ot[:, :], in_=w[:, :],
                    func=mybir.ActivationFunctionType.Copy,
                    bias=1.0 / 32.0, scale=1.0 / 16.0,
                )
                # out-DMA on the ACT HWDGE ring (qActDynamicHW) so stores
                # never contend with the in-stream for a sequencer slot.
                nc.scalar.dma_start(out=out[:, g0:g0 + g], in_=ot[:, :])
    # Bacc.finalize runs the compile pipeline (register allocation +
    # generate_event_semaphores); the pjrt exec path serializes nc.m as-is.
    nc.finalize()
    return nc


_NC = None


def _get_nc() -> bass.Bass:
    global _NC
    if _NC is None:
        _NC = _build_nc()
    return _NC


def kernel(x: np.ndarray, B=4) -> np.ndarray:
    assert int(B) == B_BITS, f"kernel hardcodes B={B_BITS}, got {B}"
    x = np.ascontiguousarray(x, dtype=np.float32)
    assert x.shape == (BATCH, COLS), x.shape
    nc = _get_nc()
    in_maps = [{"x": x[i * ROWS:(i + 1) * ROWS]} for i in range(N_CORES)]
    res = run_bass_kernel_spmd(nc, in_maps, list(range(N_CORES)))
    # Device output is bf16 (exact for these values); widen losslessly.
    return np.concatenate(
        [np.asarray(res.results[i]["out"]) for i in range(N_CORES)], axis=0
    ).astype(np.float32)
